# revision 12
# baseline (speedup 1.0000x reference)
"""Channel-attention (transposed attention) Trainium2 Bass kernel.

Reference computation (per batch b of 8, one NeuronCore each):
    xt   = x[b].reshape(C, N).T                    # [N, C], N = 64*64 = 4096
    qkv  = xt @ w_qkv                              # [N, 3C]
    q, k, v : per-head [N, hd], nh=8, hd=64
    logits_h = k_h.T @ v_h                         # [hd, hd]
    attn_h   = softmax(scale * logits_h, axis=-1)  # scale = hd**-0.5 = 1/8
    out_h    = q_h @ attn_h.T                      # [N, hd]
    y[b] = (concat_h(out_h) @ w_proj + b_proj).T   # [C, N]

Sharding: data-parallel over batch, 1 batch item per core, no collectives.

On-core layout trick: x[b] is [C, N] in DRAM, which serves both operand
orientations of the QKV projection directly:
  - k/v with tokens on partitions:  lhsT = x chunk,  rhs = w_qkv cols
  - q^T with channels on partitions: lhsT = w_qkv cols, rhs = x chunk
so no activation transpose is ever needed. The per-head [64,64] softmax
matrices are transposed on the PE via an identity matmul, packed two heads
per 128 partitions as block-diagonal matrices so the second attention
matmul and the output projection run with full 128-row contraction.

All matmuls use float32r (fp32 bytes, FP22 multiply) — 4x faster than
true fp32 at free-dim >= 256, with ~6e-5 relative element precision.
"""

import numpy as np

import concourse.bass as bass
import concourse.mybir as mybir
import concourse.tile as tile
from concourse import bass_utils

F32 = mybir.dt.float32
F32R = mybir.dt.float32r
AF = mybir.ActivationFunctionType

# Problem shape (hardcoded per contest contract).
B = 8
C = 512
H = W = 64
N = H * W            # 4096 tokens per batch
NH = 8               # heads
HD = C // NH         # 64
SCALE = HD ** -0.5   # 1/8
KC = C // 128        # 4 contraction chunks of 128 channels
NS = 8               # n-slices of 512 tokens
SL = N // NS         # 512
TT = SL // 128       # 4 token tiles of 128 per slice
HP = NH // 2         # 4 head pairs


def _r(ap):
    return ap.bitcast(F32R)


def _split_multi_waits(nc, max_waits=1):
    """The walrus build in this container encodes at most one sync-wait
    command per instruction (setupSyncWait raises "Too many sync wait
    commands" otherwise — the Tile kernel-tail drain carries several).
    Hoist excess waits onto same-engine NOPs immediately preceding the
    instruction; engine-FIFO order preserves the semantics."""
    n_split = 0
    for bb in nc.main_func.blocks:
        new_insts = []
        for ins in bb.instructions:
            si = ins.sync_info
            waits = list(si.on_wait) if si and si.on_wait else []
            if len(waits) > max_waits:
                extra, keep = waits[:-max_waits], waits[-max_waits:]
                while extra:
                    chunk, extra = extra[:max_waits], extra[max_waits:]
                    nop = mybir.InstNoOp(
                        name=nc.get_next_instruction_name(),
                        ins=[], outs=[],
                        engine=ins.engine,
                        sync_info=mybir.SyncInfo(on_wait=chunk, on_update=[]),
                    )
                    nc.register_instruction(nop)
                    new_insts.append(nop)
                    n_split += 1
                si.on_wait = keep
            new_insts.append(ins)
        bb.instructions[:] = new_insts
    return n_split


def build_nc():
    nc = bass.Bass("TRN2", debug=False, num_devices=B)

    x_t = nc.dram_tensor("x", [C, N], F32, kind="ExternalInput")
    wq_t = nc.dram_tensor("w_qkv", [C, 3 * C], F32, kind="ExternalInput")
    wp_t = nc.dram_tensor("w_proj", [C, C], F32, kind="ExternalInput")
    bp_t = nc.dram_tensor("b_proj", [C, 1], F32, kind="ExternalInput")
    y_t = nc.dram_tensor("y", [C, N], F32, kind="ExternalOutput")
    id_t = nc.inline_tensor(np.eye(128, dtype=np.float32), name="id128")

    xd, wqd, wpd, bpd, yd = x_t.ap(), wq_t.ap(), wp_t.ap(), bp_t.ap(), y_t.ap()

    with tile.TileContext(nc) as tc:
        with (
            tc.tile_pool(name="const", bufs=1) as cpool,
            tc.tile_pool(name="qt", bufs=1) as qtpool,
            tc.tile_pool(name="soft", bufs=1) as spool,
        ):
            # ---- constants / weights resident in SBUF ----
            wq_sb = [cpool.tile([128, 3 * C], F32R, name=f"wq{k}", tag=f"wq{k}") for k in range(KC)]
            wp_sb = [cpool.tile([128, C], F32R, name=f"wp{k}", tag=f"wp{k}") for k in range(KC)]
            bp_sb = [cpool.tile([128, 1], F32, name=f"bp{k}", tag=f"bp{k}") for k in range(KC)]
            id_sb = cpool.tile([128, 128], F32, tag="id")
            for k in range(KC):
                r = slice(k * 128, (k + 1) * 128)
                nc.sync.dma_start(wq_sb[k][:], _r(wqd[r, :]))
                nc.sync.dma_start(wp_sb[k][:], _r(wpd[r, :]))
                nc.sync.dma_start(bp_sb[k][:], bpd[r, :])
            nc.sync.dma_start(id_sb[:], id_t.ap()[:, :])

            # q^T, resident for the whole batch: [C, N] as 4 chunks of 128 rows
            qt_sb = [qtpool.tile([128, N], F32R, name=f"qt{k}", tag=f"qt{k}") for k in range(KC)]

            # softmax logits accumulators: heads packed 2-per-128-partitions,
            # even heads (par=0) in lg_a partitions 0:64, odd heads (par=1)
            # in lg_b partitions 64:128 (separate banks so each partition
            # half runs its own clean psum accumulation group):
            # lg_{a,b}[64*par + d, hp*64 + e] = logits of head (2*hp + par)
            with tc.tile_pool(name="lgp", bufs=1, space="PSUM") as lgpool:
                lg_a = lgpool.tile([128, HP * HD], F32, tag="lg_a")
                lg_b = lgpool.tile([128, HP * HD], F32, tag="lg_b")
                lg = [lg_a, lg_b]

                # ================= Phase A: QKV + logit accumulation ========
                with (
                    tc.tile_pool(name="xin", bufs=2) as xpool,
                    tc.tile_pool(name="kvs", bufs=2) as kvpool,
                    tc.tile_pool(name="qps", bufs=2, space="PSUM") as qpsum,
                    tc.tile_pool(name="kvp", bufs=2, space="PSUM") as kvpsum,
                ):
                    for ns in range(NS):
                        nsl = slice(ns * SL, (ns + 1) * SL)
                        xs = [xpool.tile([128, SL], F32R, name=f"x{k}", tag=f"x{k}") for k in range(KC)]
                        for k in range(KC):
                            nc.sync.dma_start(
                                xs[k][:], _r(xd[k * 128:(k + 1) * 128, nsl])
                            )
                        # --- q^T chunks: out[cq*128:, ns*512:] ---
                        for cq in range(KC):
                            qp = qpsum.tile([128, SL], F32, tag="qt_ps")
                            for k in range(KC):
                                nc.tensor.matmul(
                                    qp[:],
                                    wq_sb[k][:, cq * 128:(cq + 1) * 128],
                                    xs[k][:],
                                    start=(k == 0),
                                    stop=(k == KC - 1),
                                )
                            nc.scalar.activation(qt_sb[cq][:, nsl], qp[:], AF.Copy)
                        # --- k,v token tiles + logit accumulation ---
                        for t in range(TT):
                            kvp = kvpsum.tile([128, 2 * C], F32, tag="kv_ps")
                            for k in range(KC):
                                xk = xs[k][:, t * 128:(t + 1) * 128]
                                nc.tensor.matmul(
                                    kvp[:, 0:512], xk, wq_sb[k][:, 512:1024],
                                    start=(k == 0), stop=(k == KC - 1),
                                )
                                nc.tensor.matmul(
                                    kvp[:, 512:1024], xk, wq_sb[k][:, 1024:1536],
                                    start=(k == 0), stop=(k == KC - 1),
                                )
                            kv_sb = kvpool.tile([128, 2 * C], F32, tag="kv_sb")
                            nc.vector.tensor_copy(kv_sb[:], kvp[:])
                            first = ns == 0 and t == 0
                            last = ns == NS - 1 and t == TT - 1
                            for h in range(NH):
                                hp, par = divmod(h, 2)
                                # start=True marks the whole 2KB psum bank
                                # pending-zero, so only head 0/1 of the
                                # first tile starts each bank's group and
                                # only head 6/7 of the last tile stops it;
                                # the other heads' first write lands on
                                # still-pending bytes and overwrites.
                                nc.tensor.matmul(
                                    lg[par][par * 64:(par + 1) * 64,
                                            hp * 64:(hp + 1) * 64],
                                    kv_sb[:, h * 64:(h + 1) * 64],
                                    kv_sb[:, 512 + h * 64:512 + (h + 1) * 64],
                                    start=first and h < 2,
                                    stop=last and h >= NH - 2,
                                )

                # ================= Phase B: softmax + transposed attn =======
                # BD[hp]: block-diag exp(scale*(logits-max)) for head pair hp
                bd = [spool.tile([128, 128], F32, name=f"bd{p}", tag=f"bd{p}") for p in range(HP)]
                mx = spool.tile([128, HP], F32, tag="mx")
                bias = spool.tile([128, HP], F32, tag="bias")
                ssum = spool.tile([128, HP], F32, tag="ssum")
                recip = spool.tile([128, HP], F32, tag="recip")
                at_sb = [spool.tile([128, 128], F32R, name=f"at{p}", tag=f"at{p}") for p in range(HP)]

                for p in range(HP):
                    nc.gpsimd.memset(bd[p][:], 0.0)
                for p in range(HP):
                    csl = slice(p * 64, (p + 1) * 64)
                    for par in range(2):
                        psl = slice(par * 64, (par + 1) * 64)
                        nc.vector.reduce_max(
                            mx[psl, p:p + 1], lg[par][psl, csl],
                            axis=mybir.AxisListType.X,
                        )
                nc.vector.tensor_scalar_mul(bias[:], mx[:], -SCALE)
                for p in range(HP):
                    csl = slice(p * 64, (p + 1) * 64)
                    for par in range(2):
                        psl = slice(par * 64, (par + 1) * 64)
                        # diag block (par==0 -> cols 0:64, par==1 -> cols 64:128)
                        nc.scalar.activation(
                            bd[p][psl, psl], lg[par][psl, csl], AF.Exp,
                            bias=bias[psl, p:p + 1], scale=SCALE,
                        )
                        nc.vector.reduce_sum(
                            ssum[psl, p:p + 1], bd[p][psl, psl],
                            axis=mybir.AxisListType.X,
                        )
                nc.vector.reciprocal(recip[:], ssum[:])

            # transpose each block-diag exp matrix on the PE: at = bd^T
            with tc.tile_pool(name="bdt", bufs=2, space="PSUM") as bdtpool:
                for p in range(HP):
                    bdt = bdtpool.tile([128, 128], F32, tag="bdt")
                    nc.tensor.transpose(bdt[:], bd[p][:], id_sb[:])
                    nc.vector.tensor_copy(at_sb[p][:], bdt[:])

            # ================= Phase C+D: attention apply + projection ======
            with (
                tc.tile_pool(name="ots", bufs=2) as otpool,
                tc.tile_pool(name="ys", bufs=2) as ypool,
                tc.tile_pool(name="otp", bufs=2, space="PSUM") as otpsum,
                tc.tile_pool(name="yp", bufs=2, space="PSUM") as ypsum,
            ):
                for ns in range(NS):
                    nsl = slice(ns * SL, (ns + 1) * SL)
                    ot_sb = []
                    for p in range(HP):
                        otp = otpsum.tile([128, SL], F32, tag="ot_ps")
                        nc.tensor.matmul(
                            otp[:], at_sb[p][:], qt_sb[p][:, nsl],
                            start=True, stop=True,
                        )
                        ot = otpool.tile([128, SL], F32R, name=f"ot{p}", tag=f"ot{p}")
                        # normalization: divide head-channel row d by sum_d
                        nc.vector.tensor_scalar_mul(ot[:], otp[:], recip[:, p:p + 1])
                        ot_sb.append(ot)
                    for co in range(KC):
                        yp = ypsum.tile([128, SL], F32, tag="y_ps")
                        for k in range(KC):
                            nc.tensor.matmul(
                                yp[:],
                                wp_sb[k][:, co * 128:(co + 1) * 128],
                                ot_sb[k][:],
                                start=(k == 0),
                                stop=(k == KC - 1),
                            )
                        ysb = ypool.tile([128, SL], F32, tag="y_sb")
                        nc.scalar.activation(
                            ysb[:], yp[:], AF.Identity,
                            bias=bp_sb[co][:, 0:1], scale=1.0,
                        )
                        nc.sync.dma_start(
                            yd[co * 128:(co + 1) * 128, nsl], ysb[:]
                        )
    _split_multi_waits(nc)
    return nc


def kernel(x, w_qkv, w_proj, b_proj, num_heads):
    x = np.ascontiguousarray(np.asarray(x, dtype=np.float32))
    w_qkv = np.ascontiguousarray(np.asarray(w_qkv, dtype=np.float32))
    w_proj = np.ascontiguousarray(np.asarray(w_proj, dtype=np.float32))
    b_proj = np.ascontiguousarray(np.asarray(b_proj, dtype=np.float32))
    assert int(num_heads) == NH
    assert x.shape == (B, C, H, W)

    xs = x.reshape(B, C, N)
    bp2 = b_proj.reshape(C, 1)
    in_maps = [
        {"x": xs[b], "w_qkv": w_qkv, "w_proj": w_proj, "b_proj": bp2}
        for b in range(B)
    ]
    nc = build_nc()
    res = bass_utils.run_bass_kernel_spmd(nc, in_maps, list(range(B)))
    y = np.stack([res.results[b]["y"] for b in range(B)])
    return y.reshape(B, C, H, W).astype(np.float32)


if __name__ == "__main__":
    nc = build_nc()
    n_inst = sum(len(bb.instructions) for bb in nc.main_func.blocks)
    print(f"built OK, {n_inst} instructions")


# revision 14
# speedup vs baseline: 5.8371x; 5.8371x over previous
"""Channel-attention (transposed attention) Trainium2 Bass kernel.

Reference computation (per batch b of 8, one NeuronCore each):
    xt   = x[b].reshape(C, N).T                    # [N, C], N = 64*64 = 4096
    qkv  = xt @ w_qkv                              # [N, 3C]
    q, k, v : per-head [N, hd], nh=8, hd=64
    logits_h = k_h.T @ v_h                         # [hd, hd]
    attn_h   = softmax(scale * logits_h, axis=-1)  # scale = hd**-0.5 = 1/8
    out_h    = q_h @ attn_h.T                      # [N, hd]
    y[b] = (concat_h(out_h) @ w_proj + b_proj).T   # [C, N]

Sharding: data-parallel over batch, 1 batch item per core, no collectives.

On-core layout trick: x[b] is [C, N] in DRAM, which serves both operand
orientations of the QKV projection directly:
  - k/v with tokens on partitions:  lhsT = x chunk,  rhs = w_qkv cols
  - q^T with channels on partitions: lhsT = w_qkv cols, rhs = x chunk
so no activation transpose is ever needed. The per-head [64,64] softmax
matrices are transposed on the PE via an identity matmul, packed two heads
per 128 partitions as block-diagonal matrices so the second attention
matmul and the output projection run with full 128-row contraction.

All matmuls use float32r (fp32 bytes, FP22 multiply) — 4x faster than
true fp32 at free-dim >= 256, with ~6e-5 relative element precision.
"""

import numpy as np

import concourse.bass as bass
import concourse.mybir as mybir
import concourse.tile as tile
from concourse import bass_utils

F32 = mybir.dt.float32
F32R = mybir.dt.float32r
AF = mybir.ActivationFunctionType

# Problem shape (hardcoded per contest contract).
B = 8
C = 512
H = W = 64
N = H * W            # 4096 tokens per batch
NH = 8               # heads
HD = C // NH         # 64
SCALE = HD ** -0.5   # 1/8
KC = C // 128        # 4 contraction chunks of 128 channels
NS = 8               # n-slices of 512 tokens
SL = N // NS         # 512
TT = SL // 128       # 4 token tiles of 128 per slice
HP = NH // 2         # 4 head pairs


def _r(ap):
    return ap.bitcast(F32R)


def _split_multi_waits(nc, max_waits=1):
    """The walrus build in this container encodes at most one sync-wait
    command per instruction (setupSyncWait raises "Too many sync wait
    commands" otherwise — the Tile kernel-tail drain carries several).
    Hoist excess waits onto same-engine NOPs immediately preceding the
    instruction; engine-FIFO order preserves the semantics."""
    n_split = 0
    for bb in nc.main_func.blocks:
        new_insts = []
        for ins in bb.instructions:
            si = ins.sync_info
            waits = list(si.on_wait) if si and si.on_wait else []
            if len(waits) > max_waits:
                extra, keep = waits[:-max_waits], waits[-max_waits:]
                while extra:
                    chunk, extra = extra[:max_waits], extra[max_waits:]
                    nop = mybir.InstNoOp(
                        name=nc.get_next_instruction_name(),
                        ins=[], outs=[],
                        engine=ins.engine,
                        sync_info=mybir.SyncInfo(on_wait=chunk, on_update=[]),
                    )
                    nc.register_instruction(nop)
                    new_insts.append(nop)
                    n_split += 1
                si.on_wait = keep
            new_insts.append(ins)
        bb.instructions[:] = new_insts
    return n_split


def build_nc(reps=1):
    nc = bass.Bass("TRN2", debug=False, num_devices=B)

    x_t = nc.dram_tensor("x", [C, N], F32, kind="ExternalInput")
    wq_t = nc.dram_tensor("w_qkv", [C, 3 * C], F32, kind="ExternalInput")
    wp_t = nc.dram_tensor("w_proj", [C, C], F32, kind="ExternalInput")
    bp_t = nc.dram_tensor("b_proj", [C, 1], F32, kind="ExternalInput")
    y_t = nc.dram_tensor("y", [C, N], F32, kind="ExternalOutput")
    id_t = nc.inline_tensor(np.eye(128, dtype=np.float32), name="id128")

    xd, wqd, wpd, bpd, yd = x_t.ap(), wq_t.ap(), wp_t.ap(), bp_t.ap(), y_t.ap()

    with tile.TileContext(nc) as tc:
        with (
            tc.tile_pool(name="const", bufs=1) as cpool,
            tc.tile_pool(name="qt", bufs=1) as qtpool,
            tc.tile_pool(name="soft", bufs=1) as spool,
        ):
            # ---- constants / weights resident in SBUF ----
            wq_sb = [cpool.tile([128, 3 * C], F32R, name=f"wq{k}", tag=f"wq{k}") for k in range(KC)]
            wp_sb = [cpool.tile([128, C], F32R, name=f"wp{k}", tag=f"wp{k}") for k in range(KC)]
            bp_sb = [cpool.tile([128, 1], F32, name=f"bp{k}", tag=f"bp{k}") for k in range(KC)]
            id_sb = cpool.tile([128, 128], F32, tag="id")
            for k in range(KC):
                r = slice(k * 128, (k + 1) * 128)
                nc.sync.dma_start(wq_sb[k][:], _r(wqd[r, :]))
                nc.sync.dma_start(wp_sb[k][:], _r(wpd[r, :]))
                nc.sync.dma_start(bp_sb[k][:], bpd[r, :])
            nc.sync.dma_start(id_sb[:], id_t.ap()[:, :])

            # q^T, resident for the whole batch: [C, N] as 4 chunks of 128 rows
            qt_sb = [qtpool.tile([128, N], F32R, name=f"qt{k}", tag=f"qt{k}") for k in range(KC)]
            for _rep in range(reps):
                _build_one_pass(nc, tc, spool, wq_sb, wp_sb, bp_sb, id_sb,
                                qt_sb, xd, yd)
    _split_multi_waits(nc)
    return nc


def _build_one_pass(nc, tc, spool, wq_sb, wp_sb, bp_sb, id_sb, qt_sb, xd, yd):
    if True:
        if True:

            # softmax logits accumulators: heads packed 2-per-128-partitions,
            # even heads (par=0) in lg_a partitions 0:64, odd heads (par=1)
            # in lg_b partitions 64:128 (separate banks so each partition
            # half runs its own clean psum accumulation group):
            # lg_{a,b}[64*par + d, hp*64 + e] = logits of head (2*hp + par)
            with tc.tile_pool(name="lgp", bufs=1, space="PSUM") as lgpool:
                lg_a = lgpool.tile([128, HP * HD], F32, tag="lg_a")
                lg_b = lgpool.tile([128, HP * HD], F32, tag="lg_b")
                lg = [lg_a, lg_b]

                # ================= Phase A: QKV + logit accumulation ========
                with (
                    tc.tile_pool(name="xin", bufs=2) as xpool,
                    tc.tile_pool(name="kvs", bufs=2) as kvpool,
                    tc.tile_pool(name="qps", bufs=2, space="PSUM") as qpsum,
                    tc.tile_pool(name="kvp", bufs=2, space="PSUM") as kvpsum,
                ):
                    for ns in range(NS):
                        nsl = slice(ns * SL, (ns + 1) * SL)
                        xs = [xpool.tile([128, SL], F32R, name=f"x{k}", tag=f"x{k}") for k in range(KC)]
                        for k in range(KC):
                            nc.sync.dma_start(
                                xs[k][:], _r(xd[k * 128:(k + 1) * 128, nsl])
                            )
                        # --- q^T chunks: out[cq*128:, ns*512:] ---
                        for cq in range(KC):
                            qp = qpsum.tile([128, SL], F32, tag="qt_ps")
                            for k in range(KC):
                                nc.tensor.matmul(
                                    qp[:],
                                    wq_sb[k][:, cq * 128:(cq + 1) * 128],
                                    xs[k][:],
                                    start=(k == 0),
                                    stop=(k == KC - 1),
                                )
                            nc.scalar.activation(qt_sb[cq][:, nsl], qp[:], AF.Copy)
                        # --- k,v token tiles + logit accumulation ---
                        for t in range(TT):
                            kvp = kvpsum.tile([128, 2 * C], F32, tag="kv_ps")
                            for k in range(KC):
                                xk = xs[k][:, t * 128:(t + 1) * 128]
                                nc.tensor.matmul(
                                    kvp[:, 0:512], xk, wq_sb[k][:, 512:1024],
                                    start=(k == 0), stop=(k == KC - 1),
                                )
                                nc.tensor.matmul(
                                    kvp[:, 512:1024], xk, wq_sb[k][:, 1024:1536],
                                    start=(k == 0), stop=(k == KC - 1),
                                )
                            kv_sb = kvpool.tile([128, 2 * C], F32, tag="kv_sb")
                            nc.vector.tensor_copy(kv_sb[:], kvp[:])
                            first = ns == 0 and t == 0
                            last = ns == NS - 1 and t == TT - 1
                            for h in range(NH):
                                hp, par = divmod(h, 2)
                                # start=True marks the whole 2KB psum bank
                                # pending-zero, so only head 0/1 of the
                                # first tile starts each bank's group and
                                # only head 6/7 of the last tile stops it;
                                # the other heads' first write lands on
                                # still-pending bytes and overwrites.
                                nc.tensor.matmul(
                                    lg[par][par * 64:(par + 1) * 64,
                                            hp * 64:(hp + 1) * 64],
                                    kv_sb[:, h * 64:(h + 1) * 64],
                                    kv_sb[:, 512 + h * 64:512 + (h + 1) * 64],
                                    start=first and h < 2,
                                    stop=last and h >= NH - 2,
                                )

                # ================= Phase B: softmax + transposed attn =======
                # BD[hp]: block-diag exp(scale*(logits-max)) for head pair hp
                bd = [spool.tile([128, 128], F32, name=f"bd{p}", tag=f"bd{p}") for p in range(HP)]
                mx = spool.tile([128, HP], F32, tag="mx")
                bias = spool.tile([128, HP], F32, tag="bias")
                ssum = spool.tile([128, HP], F32, tag="ssum")
                recip = spool.tile([128, HP], F32, tag="recip")
                at_sb = [spool.tile([128, 128], F32R, name=f"at{p}", tag=f"at{p}") for p in range(HP)]

                for p in range(HP):
                    nc.gpsimd.memset(bd[p][:], 0.0)
                for p in range(HP):
                    csl = slice(p * 64, (p + 1) * 64)
                    for par in range(2):
                        psl = slice(par * 64, (par + 1) * 64)
                        nc.vector.reduce_max(
                            mx[psl, p:p + 1], lg[par][psl, csl],
                            axis=mybir.AxisListType.X,
                        )
                nc.vector.tensor_scalar_mul(bias[:], mx[:], -SCALE)
                for p in range(HP):
                    csl = slice(p * 64, (p + 1) * 64)
                    for par in range(2):
                        psl = slice(par * 64, (par + 1) * 64)
                        # diag block (par==0 -> cols 0:64, par==1 -> cols 64:128)
                        nc.scalar.activation(
                            bd[p][psl, psl], lg[par][psl, csl], AF.Exp,
                            bias=bias[psl, p:p + 1], scale=SCALE,
                        )
                        nc.vector.reduce_sum(
                            ssum[psl, p:p + 1], bd[p][psl, psl],
                            axis=mybir.AxisListType.X,
                        )
                nc.vector.reciprocal(recip[:], ssum[:])

            # transpose each block-diag exp matrix on the PE: at = bd^T
            with tc.tile_pool(name="bdt", bufs=2, space="PSUM") as bdtpool:
                for p in range(HP):
                    bdt = bdtpool.tile([128, 128], F32, tag="bdt")
                    nc.tensor.transpose(bdt[:], bd[p][:], id_sb[:])
                    nc.vector.tensor_copy(at_sb[p][:], bdt[:])

            # ================= Phase C+D: attention apply + projection ======
            with (
                tc.tile_pool(name="ots", bufs=2) as otpool,
                tc.tile_pool(name="ys", bufs=2) as ypool,
                tc.tile_pool(name="otp", bufs=2, space="PSUM") as otpsum,
                tc.tile_pool(name="yp", bufs=2, space="PSUM") as ypsum,
            ):
                for ns in range(NS):
                    nsl = slice(ns * SL, (ns + 1) * SL)
                    ot_sb = []
                    for p in range(HP):
                        otp = otpsum.tile([128, SL], F32, tag="ot_ps")
                        nc.tensor.matmul(
                            otp[:], at_sb[p][:], qt_sb[p][:, nsl],
                            start=True, stop=True,
                        )
                        ot = otpool.tile([128, SL], F32R, name=f"ot{p}", tag=f"ot{p}")
                        # normalization: divide head-channel row d by sum_d
                        nc.vector.tensor_scalar_mul(ot[:], otp[:], recip[:, p:p + 1])
                        ot_sb.append(ot)
                    for co in range(KC):
                        yp = ypsum.tile([128, SL], F32, tag="y_ps")
                        for k in range(KC):
                            nc.tensor.matmul(
                                yp[:],
                                wp_sb[k][:, co * 128:(co + 1) * 128],
                                ot_sb[k][:],
                                start=(k == 0),
                                stop=(k == KC - 1),
                            )
                        ysb = ypool.tile([128, SL], F32, tag="y_sb")
                        nc.scalar.activation(
                            ysb[:], yp[:], AF.Identity,
                            bias=bp_sb[co][:, 0:1], scale=1.0,
                        )
                        nc.sync.dma_start(
                            yd[co * 128:(co + 1) * 128, nsl], ysb[:]
                        )


def kernel(x, w_qkv, w_proj, b_proj, num_heads):
    x = np.ascontiguousarray(np.asarray(x, dtype=np.float32))
    w_qkv = np.ascontiguousarray(np.asarray(w_qkv, dtype=np.float32))
    w_proj = np.ascontiguousarray(np.asarray(w_proj, dtype=np.float32))
    b_proj = np.ascontiguousarray(np.asarray(b_proj, dtype=np.float32))
    assert int(num_heads) == NH
    assert x.shape == (B, C, H, W)

    xs = x.reshape(B, C, N)
    bp2 = b_proj.reshape(C, 1)
    in_maps = [
        {"x": xs[b], "w_qkv": w_qkv, "w_proj": w_proj, "b_proj": bp2}
        for b in range(B)
    ]
    nc = build_nc()
    res = bass_utils.run_bass_kernel_spmd(nc, in_maps, list(range(B)))
    y = np.stack([res.results[b]["y"] for b in range(B)])
    return y.reshape(B, C, H, W).astype(np.float32)


if __name__ == "__main__":
    nc = build_nc()
    n_inst = sum(len(bb.instructions) for bb in nc.main_func.blocks)
    print(f"built OK, {n_inst} instructions")


# revision 21
# speedup vs baseline: 9.8786x; 1.6924x over previous
"""Channel-attention (transposed attention) Trainium2 Bass kernel.

Reference computation (per batch b of 8, one NeuronCore each):
    xt   = x[b].reshape(C, N).T                    # [N, C], N = 64*64 = 4096
    qkv  = xt @ w_qkv                              # [N, 3C]
    q, k, v : per-head [N, hd], nh=8, hd=64
    logits_h = k_h.T @ v_h                         # [hd, hd]
    attn_h   = softmax(scale * logits_h, axis=-1)  # scale = hd**-0.5 = 1/8
    out_h    = q_h @ attn_h.T                      # [N, hd]
    y[b] = (concat_h(out_h) @ w_proj + b_proj).T   # [C, N]

Sharding: data-parallel over batch, 1 batch item per core, no collectives.

On-core layout trick: x[b] is [C, N] in DRAM, which serves both operand
orientations of the QKV projection directly:
  - k/v with tokens on partitions:  lhsT = x chunk,  rhs = w_qkv cols
  - q^T with channels on partitions: lhsT = w_qkv cols, rhs = x chunk
so no activation transpose is ever needed. The per-head [64,64] softmax
matrices are transposed on the PE via an identity matmul, packed two heads
per 128 partitions as block-diagonal matrices so the second attention
matmul and the output projection run with full 128-row contraction.

The large (free-dim 512) matmuls use float32r (fp32 bytes, FP22
multiply) — 4x faster than true fp32 at free-dim >= 256, ~6e-5 relative
element precision. The small per-head logit matmuls (free-dim 64, where
f32r has no speed edge) stay exact fp32, which also permits the
tile_position col-64 packing that runs odd heads into partitions 64:128.
"""

import numpy as np

import concourse.bass as bass
import concourse.mybir as mybir
import concourse.tile as tile
from concourse import bass_utils

F32 = mybir.dt.float32
F32R = mybir.dt.float32r
AF = mybir.ActivationFunctionType

# Problem shape (hardcoded per contest contract).
B = 8
C = 512
H = W = 64
N = H * W            # 4096 tokens per batch
NH = 8               # heads
HD = C // NH         # 64
SCALE = HD ** -0.5   # 1/8
KC = C // 128        # 4 contraction chunks of 128 channels
NS = 8               # n-slices of 512 tokens
SL = N // NS         # 512
TT = SL // 128       # 4 token tiles of 128 per slice
HP = NH // 2         # 4 head pairs


def _r(ap):
    return ap.bitcast(F32R)


def _split_multi_waits(nc, max_waits=1):
    """The walrus build in this container encodes at most one sync-wait
    command per instruction (setupSyncWait raises "Too many sync wait
    commands" otherwise — the Tile kernel-tail drain carries several).
    Hoist excess waits onto same-engine NOPs immediately preceding the
    instruction; engine-FIFO order preserves the semantics."""
    n_split = 0
    for bb in nc.main_func.blocks:
        new_insts = []
        for ins in bb.instructions:
            si = ins.sync_info
            waits = list(si.on_wait) if si and si.on_wait else []
            if len(waits) > max_waits:
                extra, keep = waits[:-max_waits], waits[-max_waits:]
                while extra:
                    chunk, extra = extra[:max_waits], extra[max_waits:]
                    nop = mybir.InstNoOp(
                        name=nc.get_next_instruction_name(),
                        ins=[], outs=[],
                        engine=ins.engine,
                        sync_info=mybir.SyncInfo(on_wait=chunk, on_update=[]),
                    )
                    nc.register_instruction(nop)
                    new_insts.append(nop)
                    n_split += 1
                si.on_wait = keep
            new_insts.append(ins)
        bb.instructions[:] = new_insts
    return n_split


def build_nc(reps=1, phases='full'):
    nc = bass.Bass("TRN2", debug=False, num_devices=B)

    x_t = nc.dram_tensor("x", [C, N], F32, kind="ExternalInput")
    wq_t = nc.dram_tensor("w_qkv", [C, 3 * C], F32, kind="ExternalInput")
    wp_t = nc.dram_tensor("w_proj", [C, C], F32, kind="ExternalInput")
    bp_t = nc.dram_tensor("b_proj", [C, 1], F32, kind="ExternalInput")
    y_t = nc.dram_tensor("y", [C, N], F32, kind="ExternalOutput")
    id_t = nc.inline_tensor(np.eye(128, dtype=np.float32), name="id128")

    xd, wqd, wpd, bpd, yd = x_t.ap(), wq_t.ap(), wp_t.ap(), bp_t.ap(), y_t.ap()

    with tile.TileContext(nc) as tc:
        with (
            tc.tile_pool(name="const", bufs=1) as cpool,
            tc.tile_pool(name="qt", bufs=1) as qtpool,
            tc.tile_pool(name="soft", bufs=1) as spool,
        ):
            # ---- constants / weights resident in SBUF ----
            wq_sb = [cpool.tile([128, 3 * C], F32R, name=f"wq{k}", tag=f"wq{k}") for k in range(KC)]
            wp_sb = [cpool.tile([128, C], F32R, name=f"wp{k}", tag=f"wp{k}") for k in range(KC)]
            bp_sb = [cpool.tile([128, 1], F32, name=f"bp{k}", tag=f"bp{k}") for k in range(KC)]
            id_sb = cpool.tile([128, 128], F32, tag="id")
            for k in range(KC):
                r = slice(k * 128, (k + 1) * 128)
                nc.sync.dma_start(wq_sb[k][:], _r(wqd[r, :]))
            nc.sync.dma_start(id_sb[:], id_t.ap()[:, :])

            # q^T, resident for the whole batch: [C, N] as 4 chunks of 128 rows
            qt_sb = [qtpool.tile([128, N], F32R, name=f"qt{k}", tag=f"qt{k}") for k in range(KC)]
            for _rep in range(reps):
                _build_one_pass(nc, tc, spool, wq_sb, wp_sb, bp_sb, id_sb,
                                qt_sb, xd, yd, wpd, bpd, first_rep=(_rep == 0),
                                phases=phases)
    _split_multi_waits(nc)
    return nc


def _build_one_pass(nc, tc, spool, wq_sb, wp_sb, bp_sb, id_sb, qt_sb, xd, yd,
                    wpd, bpd, first_rep=True, phases="full"):
    # phases: prefix gating for attribution benchmarks
    lvl = ["dma", "qkv", "logits", "soft", "attn", "full"].index(phases)
    if True:
        if True:

            # softmax logits accumulators: heads packed 2-per-128-partitions,
            # even heads (par=0) in lg_a partitions 0:64, odd heads (par=1)
            # in lg_b partitions 64:128 (separate banks so each partition
            # half runs its own clean psum accumulation group):
            # lg_{a,b}[64*par + d, hp*64 + e] = logits of head (2*hp + par)
            with tc.tile_pool(name="lgp", bufs=1, space="PSUM") as lgpool:
                lg_a = lgpool.tile([128, HP * HD], F32, tag="lg_a")
                lg_b = lgpool.tile([128, HP * HD], F32, tag="lg_b")
                lg = [lg_a, lg_b]

                # ================= Phase A: QKV + logit accumulation ========
                with (
                    tc.tile_pool(name="xin", bufs=2) as xpool,
                    tc.tile_pool(name="kvs", bufs=2) as kvpool,
                    tc.tile_pool(name="qps", bufs=2, space="PSUM") as qpsum,
                    tc.tile_pool(name="kvp", bufs=2, space="PSUM") as kvpsum,
                ):
                    for ns in range(NS):
                        nsl = slice(ns * SL, (ns + 1) * SL)
                        xs = [xpool.tile([128, SL], F32R, name=f"x{k}", tag=f"x{k}") for k in range(KC)]
                        for k in range(KC):
                            nc.sync.dma_start(
                                xs[k][:], _r(xd[k * 128:(k + 1) * 128, nsl])
                            )
                        if lvl < 1:
                            continue
                        # --- q^T chunks: out[cq*128:, ns*512:] ---
                        for cq in range(KC):
                            qp = qpsum.tile([128, SL], F32, tag="qt_ps")
                            for k in range(KC):
                                nc.tensor.matmul(
                                    qp[:],
                                    wq_sb[k][:, cq * 128:(cq + 1) * 128],
                                    xs[k][:],
                                    start=(k == 0),
                                    stop=(k == KC - 1),
                                )
                            nc.scalar.activation(qt_sb[cq][:, nsl], qp[:], AF.Copy)
                        # --- k,v token tiles + logit accumulation ---
                        for t in range(TT):
                            kvp = kvpsum.tile([128, 2 * C], F32, tag="kv_ps")
                            for k in range(KC):
                                xk = xs[k][:, t * 128:(t + 1) * 128]
                                nc.tensor.matmul(
                                    kvp[:, 0:512], xk, wq_sb[k][:, 512:1024],
                                    start=(k == 0), stop=(k == KC - 1),
                                )
                                nc.tensor.matmul(
                                    kvp[:, 512:1024], xk, wq_sb[k][:, 1024:1536],
                                    start=(k == 0), stop=(k == KC - 1),
                                )
                            kv_sb = kvpool.tile([128, 2 * C], F32, tag="kv_sb")
                            nc.vector.tensor_copy(kv_sb[:], kvp[:])
                            if lvl < 2:
                                continue
                            first = ns == 0 and t == 0
                            last = ns == NS - 1 and t == TT - 1
                            for h in range(NH):
                                hp, par = divmod(h, 2)
                                # start=True marks the whole 2KB psum bank
                                # pending-zero, so only head 0/1 of the
                                # first tile starts each bank's group and
                                # only head 6/7 of the last tile stops it;
                                # the other heads' first write lands on
                                # still-pending bytes and overwrites.
                                nc.tensor.matmul(
                                    lg[par][par * 64:(par + 1) * 64,
                                            hp * 64:(hp + 1) * 64],
                                    kv_sb[:, h * 64:(h + 1) * 64],
                                    kv_sb[:, 512 + h * 64:512 + (h + 1) * 64],
                                    start=first and h < 2,
                                    stop=last and h >= NH - 2,
                                )

                # deferred weight loads: w_proj/b_proj are first needed in
                # phase C/D, so their DMAs stay off the startup critical path
                if first_rep:
                    for k in range(KC):
                        r = slice(k * 128, (k + 1) * 128)
                        nc.sync.dma_start(wp_sb[k][:], _r(wpd[r, :]))
                        nc.sync.dma_start(bp_sb[k][:], bpd[r, :])

                if lvl < 3:
                    return
                # ================= Phase B: softmax + transposed attn =======
                # BD[hp]: block-diag exp(scale*(logits-max)) for head pair hp
                bd = [spool.tile([128, 128], F32, name=f"bd{p}", tag=f"bd{p}") for p in range(HP)]
                mx = spool.tile([128, HP], F32, tag="mx")
                bias = spool.tile([128, HP], F32, tag="bias")
                ssum = spool.tile([128, HP], F32, tag="ssum")
                recip = spool.tile([128, HP], F32, tag="recip")
                at_sb = [spool.tile([128, 128], F32R, name=f"at{p}", tag=f"at{p}") for p in range(HP)]

                for p in range(HP):
                    nc.gpsimd.memset(bd[p][:], 0.0)
                for p in range(HP):
                    csl = slice(p * 64, (p + 1) * 64)
                    for par in range(2):
                        psl = slice(par * 64, (par + 1) * 64)
                        nc.vector.reduce_max(
                            mx[psl, p:p + 1], lg[par][psl, csl],
                            axis=mybir.AxisListType.X,
                        )
                nc.vector.tensor_scalar_mul(bias[:], mx[:], -SCALE)
                for p in range(HP):
                    csl = slice(p * 64, (p + 1) * 64)
                    for par in range(2):
                        psl = slice(par * 64, (par + 1) * 64)
                        # diag block (par==0 -> cols 0:64, par==1 -> cols 64:128)
                        nc.scalar.activation(
                            bd[p][psl, psl], lg[par][psl, csl], AF.Exp,
                            bias=bias[psl, p:p + 1], scale=SCALE,
                        )
                        nc.vector.reduce_sum(
                            ssum[psl, p:p + 1], bd[p][psl, psl],
                            axis=mybir.AxisListType.X,
                        )
                nc.vector.reciprocal(recip[:], ssum[:])

            # transpose each block-diag exp matrix on the PE: at = bd^T
            with tc.tile_pool(name="bdt", bufs=2, space="PSUM") as bdtpool:
                for p in range(HP):
                    bdt = bdtpool.tile([128, 128], F32, tag="bdt")
                    nc.tensor.transpose(bdt[:], bd[p][:], id_sb[:])
                    nc.vector.tensor_copy(at_sb[p][:], bdt[:])

            if lvl < 4:
                return
            # ================= Phase C+D: attention apply + projection ======
            with (
                tc.tile_pool(name="ots", bufs=2) as otpool,
                tc.tile_pool(name="ys", bufs=3) as ypool,
                tc.tile_pool(name="otp", bufs=4, space="PSUM") as otpsum,
                tc.tile_pool(name="yp", bufs=3, space="PSUM") as ypsum,
            ):
                for ns in range(NS):
                    nsl = slice(ns * SL, (ns + 1) * SL)
                    ot_sb = []
                    for p in range(HP):
                        otp = otpsum.tile([128, SL], F32, tag="ot_ps")
                        nc.tensor.matmul(
                            otp[:], at_sb[p][:], qt_sb[p][:, nsl],
                            start=True, stop=True,
                        )
                        ot = otpool.tile([128, SL], F32R, name=f"ot{p}", tag=f"ot{p}")
                        # normalization: divide head-channel row d by sum_d
                        nc.vector.tensor_scalar_mul(ot[:], otp[:], recip[:, p:p + 1])
                        ot_sb.append(ot)
                    if lvl < 5:
                        continue
                    for co in range(KC):
                        yp = ypsum.tile([128, SL], F32, tag="y_ps")
                        for k in range(KC):
                            nc.tensor.matmul(
                                yp[:],
                                wp_sb[k][:, co * 128:(co + 1) * 128],
                                ot_sb[k][:],
                                start=(k == 0),
                                stop=(k == KC - 1),
                            )
                        ysb = ypool.tile([128, SL], F32, tag="y_sb")
                        nc.scalar.activation(
                            ysb[:], yp[:], AF.Identity,
                            bias=bp_sb[co][:, 0:1], scale=1.0,
                        )
                        nc.sync.dma_start(
                            yd[co * 128:(co + 1) * 128, nsl], ysb[:]
                        )


_NC_CACHE = None


def kernel(x, w_qkv, w_proj, b_proj, num_heads):
    x = np.ascontiguousarray(np.asarray(x, dtype=np.float32))
    w_qkv = np.ascontiguousarray(np.asarray(w_qkv, dtype=np.float32))
    w_proj = np.ascontiguousarray(np.asarray(w_proj, dtype=np.float32))
    b_proj = np.ascontiguousarray(np.asarray(b_proj, dtype=np.float32))
    assert int(num_heads) == NH
    assert x.shape == (B, C, H, W)

    xs = x.reshape(B, C, N)
    bp2 = b_proj.reshape(C, 1)
    in_maps = [
        {"x": xs[b], "w_qkv": w_qkv, "w_proj": w_proj, "b_proj": bp2}
        for b in range(B)
    ]
    global _NC_CACHE
    if _NC_CACHE is None:
        _NC_CACHE = build_nc()
    res = bass_utils.run_bass_kernel_spmd(_NC_CACHE, in_maps, list(range(B)))
    y = np.stack([res.results[b]["y"] for b in range(B)])
    return y.reshape(B, C, H, W).astype(np.float32)


if __name__ == "__main__":
    nc = build_nc()
    n_inst = sum(len(bb.instructions) for bb in nc.main_func.blocks)
    print(f"built OK, {n_inst} instructions")
